# revision 4
# baseline (speedup 1.0000x reference)
"""Causal self-attention (B=4, T=4096, D=1024, fp32) on 8 trn2 NeuronCores.

Sharding: 2 cores per batch. Within a batch, core h in {0,1} owns the
key blocks of parity h (128-wide blocks at global positions 2j+h). Each
core computes, for ALL queries of its batch, the unnormalized partial
attention output restricted to its own keys, already pushed through the
output projection, plus the partial softmax denominators:

    outT_h = W_o @ (sum_{k in parity h, k<=q} exp(s_qk) * v_k)^T
    denom_h[q] = sum_{k in parity h, k<=q} exp(s_qk)

Because row scaling commutes with the right-side matmul, the host merge
is exact:  out[q] = (outT_0[:,q] + outT_1[:,q]) / (denom_0[q] + denom_1[q]).

Softmax is computed without max subtraction (scores are ~N(0,1) here, so
exp never overflows in fp32), which makes the partial-denominator merge
trivial.

Every core runs the same program: q-tile i (256 queries) attends to its
i+1 local key blocks; the parity-dependent diagonal mask and the
gathered key tokens arrive as data.

All matmuls are bf16 x bf16 with fp32 PSUM accumulation (full PE rate).
Measured model error vs the fp32 reference: ~3e-3 scale-relative absmax.
"""

import sys

if "/opt/trn_rl_repo" not in sys.path:
    sys.path.insert(0, "/opt/trn_rl_repo")

import numpy as np
import ml_dtypes

BF16 = ml_dtypes.bfloat16

D = 1024
P = 128          # partition / contraction block
DB = D // P      # 8 d-blocks

_PROGRAM_CACHE = {}


def build_program(T, TQ):
    """Build + compile the single-core SPMD program. Returns the Bacc."""
    import concourse.mybir as mybir
    import concourse.tile as tile
    from concourse import bacc

    bf = mybir.dt.bfloat16
    f32 = mybir.dt.float32

    NT = T // TQ             # q-tiles per core (16)
    KB_PER_TILE = TQ // (2 * P)  # local key blocks added per q-tile (1)
    assert TQ == 2 * P, "program assumes one local key block per q-tile step"
    TKV = T // 2             # parity keys per core (2048)
    NKB = TKV // P           # local key blocks (16)
    KV_TT = 512              # token tile for the K/V projection phase
    NKVT = TKV // KV_TT      # 4

    nc = bacc.Bacc("TRN2", target_bir_lowering=False, debug=False, num_devices=8)

    xT = nc.dram_tensor("xT", [D, T], bf, kind="ExternalInput")
    xT_kv = nc.dram_tensor("xT_kv", [D, TKV], bf, kind="ExternalInput")
    w_qT = nc.dram_tensor("w_qT", [D, D], bf, kind="ExternalInput")
    w_kT = nc.dram_tensor("w_kT", [D, D], bf, kind="ExternalInput")
    w_vT = nc.dram_tensor("w_vT", [D, D], bf, kind="ExternalInput")
    w_oT = nc.dram_tensor("w_oT", [D, D], bf, kind="ExternalInput")
    mask = nc.dram_tensor("mask", [P, TQ], bf, kind="ExternalInput")
    outT = nc.dram_tensor("outT", [D, T], f32, kind="ExternalOutput")
    denom = nc.dram_tensor("denom", [NT, TQ], f32, kind="ExternalOutput")

    xT_r = xT.rearrange("(po pi) t -> pi po t", pi=P)
    xT_kv_r = xT_kv.rearrange("(po pi) t -> pi po t", pi=P)
    w_qT_r = w_qT.rearrange("(po pi) f -> pi po f", pi=P)
    w_kT_r = w_kT.rearrange("(po pi) f -> pi po f", pi=P)
    w_vT_r = w_vT.rearrange("(po pi) f -> pi po f", pi=P)
    w_oT_r = w_oT.rearrange("(po pi) f -> pi po f", pi=P)
    outT_r = outT.rearrange("(po pi) t -> pi po t", pi=P)

    with tile.TileContext(nc) as tc:
        with tc.tile_pool(name="res", bufs=1) as res:
            # Persistent SBUF: K^T (d-major), V (token-major), W_q, W_o, mask, ones
            kT_sb = res.tile([P, DB, TKV], bf)
            v_sb = res.tile([P, NKB, D], bf)
            wq_sb = res.tile([P, DB, D], bf)
            wo_sb = res.tile([P, DB, D], bf)
            mask_sb = res.tile([P, TQ], bf)
            ones_sb = res.tile([P, 1], bf)

            nc.sync.dma_start(wq_sb[:], w_qT_r[:])
            nc.sync.dma_start(wo_sb[:], w_oT_r[:])
            nc.sync.dma_start(mask_sb[:], mask[:])
            nc.vector.memset(ones_sb[:], 1.0)

            # ---- Phase A: K/V projection of the parity keys ----
            with tc.tile_pool(name="pa_sb", bufs=2) as pa_sb, \
                 tc.tile_pool(name="pa_w", bufs=1) as pa_w, \
                 tc.tile_pool(name="pa_ps", bufs=2, space="PSUM") as pa_ps:
                wk_sb = pa_w.tile([P, DB, D], bf)
                wv_sb = pa_w.tile([P, DB, D], bf)
                nc.sync.dma_start(wk_sb[:], w_kT_r[:])
                nc.sync.dma_start(wv_sb[:], w_vT_r[:])

                for tt in range(NKVT):
                    xkv = pa_sb.tile([P, DB, KV_TT], bf, tag="xkv")
                    nc.sync.dma_start(
                        xkv[:], xT_kv_r[:, :, tt * KV_TT:(tt + 1) * KV_TT])
                    # K^T[dout, tok] += W_k^T[din, dout].T @ x^T[din, tok]
                    for do in range(DB):
                        kps = pa_ps.tile([P, KV_TT], f32, tag="kps")
                        for di in range(DB):
                            nc.tensor.matmul(
                                kps[:],
                                wk_sb[:, di, do * P:(do + 1) * P],
                                xkv[:, di, :],
                                start=(di == 0), stop=(di == DB - 1))
                        nc.vector.tensor_copy(
                            kT_sb[:, do, tt * KV_TT:(tt + 1) * KV_TT], kps[:])
                    # V[tok, dout] += x^T[din, tok].T @ W_v^T[din, dout]
                    for tb in range(KV_TT // P):
                        for dh in range(D // 512):
                            vps = pa_ps.tile([P, 512], f32, tag="vps")
                            for di in range(DB):
                                nc.tensor.matmul(
                                    vps[:],
                                    xkv[:, di, tb * P:(tb + 1) * P],
                                    wv_sb[:, di, dh * 512:(dh + 1) * 512],
                                    start=(di == 0), stop=(di == DB - 1))
                            nc.vector.tensor_copy(
                                v_sb[:, tt * (KV_TT // P) + tb,
                                     dh * 512:(dh + 1) * 512], vps[:])

            # ---- Phase B: per q-tile attention + output projection ----
            with tc.tile_pool(name="pb_sb", bufs=2) as pb_sb, \
                 tc.tile_pool(name="pb_pan", bufs=2) as pb_pan, \
                 tc.tile_pool(name="mm_ps", bufs=2, space="PSUM") as mm_ps, \
                 tc.tile_pool(name="s_ps", bufs=2, space="PSUM") as s_ps, \
                 tc.tile_pool(name="y_ps", bufs=2, space="PSUM") as y_ps, \
                 tc.tile_pool(name="d_ps", bufs=1, space="PSUM") as d_ps:
                for i in range(NT):
                    nkb = i + 1  # local key blocks for this q-tile
                    q0 = i * TQ

                    # Q projection (scale folded into w_qT host-side).
                    # Per-po DMAs: a single 3-D DMA would need a 128*T*2B
                    # middle-dim stride (1 MiB at T=4096), which overflows a
                    # descriptor stride field and faults the device.
                    xq = pb_sb.tile([P, DB, TQ], bf, tag="xq")
                    for po in range(DB):
                        nc.sync.dma_start(
                            xq[:, po, :], xT_r[:, po, q0:q0 + TQ])
                    qT = pb_sb.tile([P, DB, TQ], bf, tag="qT")
                    for do in range(DB):
                        qps = mm_ps.tile([P, TQ], f32, tag="mm")
                        for di in range(DB):
                            nc.tensor.matmul(
                                qps[:],
                                wq_sb[:, di, do * P:(do + 1) * P],
                                xq[:, di, :],
                                start=(di == 0), stop=(di == DB - 1))
                        nc.vector.tensor_copy(qT[:, do, :], qps[:])

                    # S^T blocks -> exp -> (mask) -> panel; denominators
                    panel = pb_pan.tile([P, NT, TQ], bf, tag="panel")
                    dps = d_ps.tile([1, TQ], f32, tag="den")
                    for j in range(nkb):
                        sps = s_ps.tile([P, TQ], f32, tag="s")
                        for di in range(DB):
                            nc.tensor.matmul(
                                sps[:],
                                kT_sb[:, di, j * P:(j + 1) * P],
                                qT[:, di, :],
                                start=(di == 0), stop=(di == DB - 1))
                        nc.scalar.activation(
                            panel[:, j, :], sps[:],
                            mybir.ActivationFunctionType.Exp)
                        if j == nkb - 1:
                            nc.vector.tensor_mul(
                                out=panel[:, j, :], in0=panel[:, j, :],
                                in1=mask_sb[:])
                        nc.tensor.matmul(
                            dps[:], ones_sb[:], panel[:, j, :],
                            start=(j == 0), stop=(j == nkb - 1))
                    dstage = pb_sb.tile([1, TQ], f32, tag="dstage")
                    nc.vector.tensor_copy(dstage[:], dps[:])
                    nc.sync.dma_start(denom[i:i + 1, :], dstage[0:1, :])

                    # y^T[dout, q] += V[k, dout].T @ expS^T[k, q]
                    yT = pb_sb.tile([P, DB, TQ], bf, tag="yT")
                    for do in range(DB):
                        yps = y_ps.tile([P, TQ], f32, tag="y")
                        for j in range(nkb):
                            nc.tensor.matmul(
                                yps[:],
                                v_sb[:, j, do * P:(do + 1) * P],
                                panel[:, j, :],
                                start=(j == 0), stop=(j == nkb - 1))
                        nc.vector.tensor_copy(yT[:, do, :], yps[:])

                    # out^T[dout, q] += W_o^T[din, dout].T @ y^T[din, q]
                    for do in range(DB):
                        ops = mm_ps.tile([P, TQ], f32, tag="mm")
                        for di in range(DB):
                            nc.tensor.matmul(
                                ops[:],
                                wo_sb[:, di, do * P:(do + 1) * P],
                                yT[:, di, :],
                                start=(di == 0), stop=(di == DB - 1))
                        ostage = pb_sb.tile([P, TQ], f32, tag="ostage")
                        nc.vector.tensor_copy(ostage[:], ops[:])
                        nc.sync.dma_start(outT_r[:, do, q0:q0 + TQ], ostage[:])

    nc.compile()
    return nc


def _prepare_core_inputs(x, W_q, W_k, W_v, W_o, T, TQ):
    """Host-side shard prep. Returns list of 8 in_maps (bf16 ndarrays)."""
    B = x.shape[0]
    scale = 1.0 / np.sqrt(np.float32(D))

    w_qT = np.ascontiguousarray((W_q.T * scale)).astype(BF16)
    w_kT = np.ascontiguousarray(W_k.T).astype(BF16)
    w_vT = np.ascontiguousarray(W_v.T).astype(BF16)
    w_oT = np.ascontiguousarray(W_o.T).astype(BF16)

    # Diagonal masks per parity: mask[k, q] = 1 if k + 128*h <= q
    k_idx = np.arange(P)[:, None]
    q_idx = np.arange(TQ)[None, :]
    masks = [
        (k_idx + P * h <= q_idx).astype(np.float32).astype(BF16)
        for h in (0, 1)
    ]

    in_maps = []
    for b in range(B):
        xb = x[b]                                   # [T, D] fp32
        xT = np.ascontiguousarray(xb.T).astype(BF16)  # [D, T]
        # parity gather of 128-wide key blocks
        xblk = xT.reshape(D, T // (2 * P), 2, P)      # [D, n, parity, 128]
        for h in (0, 1):
            xT_kv = np.ascontiguousarray(
                xblk[:, :, h, :].reshape(D, T // 2))
            in_maps.append({
                "xT": xT, "xT_kv": xT_kv,
                "w_qT": w_qT, "w_kT": w_kT, "w_vT": w_vT, "w_oT": w_oT,
                "mask": masks[h],
            })
    return in_maps


def _merge(results, B, T):
    """Host merge: (out0+out1)/(d0+d1) per batch, back to [B, T, D] fp32."""
    out = np.empty((B, T, D), dtype=np.float32)
    for b in range(B):
        o0 = results[2 * b]["outT"]
        o1 = results[2 * b + 1]["outT"]
        d0 = results[2 * b]["denom"].reshape(T)
        d1 = results[2 * b + 1]["denom"].reshape(T)
        out[b] = ((o0 + o1) / (d0 + d1)[None, :]).T
    return out


def kernel(x, W_q, W_k, W_v, W_o):
    from concourse.bass_utils import run_bass_kernel_spmd

    x = np.asarray(x)
    B, T, d = x.shape
    assert d == D
    TQ = 256

    key = (T, TQ)
    if key not in _PROGRAM_CACHE:
        _PROGRAM_CACHE[key] = build_program(T, TQ)
    nc = _PROGRAM_CACHE[key]

    in_maps = _prepare_core_inputs(
        np.asarray(x, np.float32), np.asarray(W_q, np.float32),
        np.asarray(W_k, np.float32), np.asarray(W_v, np.float32),
        np.asarray(W_o, np.float32), T, TQ)
    res = run_bass_kernel_spmd(nc, in_maps, list(range(2 * B)))
    return _merge(res.results, B, T)


# revision 6
# speedup vs baseline: 1.0203x; 1.0203x over previous
"""Causal self-attention (B=4, T=4096, D=1024, fp32) on 8 trn2 NeuronCores.

Sharding: 2 cores per batch. Within a batch, core h in {0,1} owns the
key blocks of parity h (128-wide blocks at global positions 2j+h). Each
core computes, for ALL queries of its batch, the unnormalized partial
attention output restricted to its own keys, already pushed through the
output projection, plus the partial softmax denominators:

    outT_h = W_o @ (sum_{k in parity h, k<=q} exp(s_qk) * v_k)^T
    denom_h[q] = sum_{k in parity h, k<=q} exp(s_qk)

Because row scaling commutes with the right-side matmul, the host merge
is exact:  out[q] = (outT_0[:,q] + outT_1[:,q]) / (denom_0[q] + denom_1[q]).

Softmax is computed without max subtraction (scores are ~N(0,1) here, so
exp never overflows in fp32), which makes the partial-denominator merge
trivial.

Every core runs the same program: q-tile i (256 queries) attends to its
i+1 local key blocks; the parity-dependent diagonal mask and the
gathered key tokens arrive as data.

All matmuls are bf16 x bf16 with fp32 PSUM accumulation (full PE rate).
Measured model error vs the fp32 reference: ~3e-3 scale-relative absmax.
"""

import sys

if "/opt/trn_rl_repo" not in sys.path:
    sys.path.insert(0, "/opt/trn_rl_repo")

import numpy as np
import ml_dtypes

BF16 = ml_dtypes.bfloat16

D = 1024
P = 128          # partition / contraction block
DB = D // P      # 8 d-blocks

_PROGRAM_CACHE = {}


def build_program(T, TQ):
    """Build + compile the single-core SPMD program. Returns the Bacc."""
    import concourse.mybir as mybir
    import concourse.tile as tile
    from concourse import bacc

    bf = mybir.dt.bfloat16
    f32 = mybir.dt.float32

    NT = T // TQ             # q-tiles per core (16)
    KB_PER_TILE = TQ // (2 * P)  # local key blocks added per q-tile (1)
    assert TQ == 2 * P, "program assumes one local key block per q-tile step"
    TKV = T // 2             # parity keys per core (2048)
    NKB = TKV // P           # local key blocks (16)
    KV_TT = 512              # token tile for the K/V projection phase
    NKVT = TKV // KV_TT      # 4

    nc = bacc.Bacc("TRN2", target_bir_lowering=False, debug=False, num_devices=8)

    xT = nc.dram_tensor("xT", [D, T], bf, kind="ExternalInput")
    xT_kv = nc.dram_tensor("xT_kv", [D, TKV], bf, kind="ExternalInput")
    w_qT = nc.dram_tensor("w_qT", [D, D], bf, kind="ExternalInput")
    w_kT = nc.dram_tensor("w_kT", [D, D], bf, kind="ExternalInput")
    w_vT = nc.dram_tensor("w_vT", [D, D], bf, kind="ExternalInput")
    w_oT = nc.dram_tensor("w_oT", [D, D], bf, kind="ExternalInput")
    mask = nc.dram_tensor("mask", [P, TQ], bf, kind="ExternalInput")
    outT = nc.dram_tensor("outT", [D, T], f32, kind="ExternalOutput")
    denom = nc.dram_tensor("denom", [NT, TQ], f32, kind="ExternalOutput")

    xT_r = xT.rearrange("(po pi) t -> pi po t", pi=P)
    xT_kv_r = xT_kv.rearrange("(po pi) t -> pi po t", pi=P)
    w_qT_r = w_qT.rearrange("(po pi) f -> pi po f", pi=P)
    w_kT_r = w_kT.rearrange("(po pi) f -> pi po f", pi=P)
    w_vT_r = w_vT.rearrange("(po pi) f -> pi po f", pi=P)
    w_oT_r = w_oT.rearrange("(po pi) f -> pi po f", pi=P)
    outT_r = outT.rearrange("(po pi) t -> pi po t", pi=P)

    with tile.TileContext(nc) as tc:
        with tc.tile_pool(name="res", bufs=1) as res:
            # Persistent SBUF: K^T (d-major), V (token-major), W_q, W_o, mask, ones
            kT_sb = res.tile([P, DB, TKV], bf)
            v_sb = res.tile([P, NKB, D], bf)
            wq_sb = res.tile([P, DB, D], bf)
            wo_sb = res.tile([P, DB, D], bf)
            mask_sb = res.tile([P, TQ], bf)
            ones_sb = res.tile([P, 1], bf)

            nc.vector.memset(ones_sb[:], 1.0)

            # ---- Phase A: K/V projection of the parity keys ----
            with tc.tile_pool(name="pa_sb", bufs=2) as pa_sb, \
                 tc.tile_pool(name="pa_w", bufs=1) as pa_w, \
                 tc.tile_pool(name="pa_ps", bufs=2, space="PSUM") as pa_ps:
                wk_sb = pa_w.tile([P, DB, D], bf)
                wv_sb = pa_w.tile([P, DB, D], bf)
                # phase-A-critical loads first: PE's first matmul waits on
                # wk + the first xkv tile; wq/wo/mask can trickle in later.
                xkv0 = pa_sb.tile([P, DB, KV_TT], bf, tag="xkv")
                nc.sync.dma_start(wk_sb[:], w_kT_r[:])
                nc.sync.dma_start(xkv0[:], xT_kv_r[:, :, 0:KV_TT])
                nc.sync.dma_start(wv_sb[:], w_vT_r[:])
                nc.sync.dma_start(wq_sb[:], w_qT_r[:])
                nc.sync.dma_start(wo_sb[:], w_oT_r[:])
                nc.sync.dma_start(mask_sb[:], mask[:])

                for tt in range(NKVT):
                    if tt == 0:
                        xkv = xkv0
                    else:
                        xkv = pa_sb.tile([P, DB, KV_TT], bf, tag="xkv")
                        nc.sync.dma_start(
                            xkv[:], xT_kv_r[:, :, tt * KV_TT:(tt + 1) * KV_TT])
                    # K^T[dout, tok] += W_k^T[din, dout].T @ x^T[din, tok]
                    for do in range(DB):
                        kps = pa_ps.tile([P, KV_TT], f32, tag="kps")
                        for di in range(DB):
                            nc.tensor.matmul(
                                kps[:],
                                wk_sb[:, di, do * P:(do + 1) * P],
                                xkv[:, di, :],
                                start=(di == 0), stop=(di == DB - 1))
                        nc.vector.tensor_copy(
                            kT_sb[:, do, tt * KV_TT:(tt + 1) * KV_TT], kps[:])
                    # V[tok, dout] += x^T[din, tok].T @ W_v^T[din, dout]
                    for tb in range(KV_TT // P):
                        for dh in range(D // 512):
                            vps = pa_ps.tile([P, 512], f32, tag="vps")
                            for di in range(DB):
                                nc.tensor.matmul(
                                    vps[:],
                                    xkv[:, di, tb * P:(tb + 1) * P],
                                    wv_sb[:, di, dh * 512:(dh + 1) * 512],
                                    start=(di == 0), stop=(di == DB - 1))
                            nc.vector.tensor_copy(
                                v_sb[:, tt * (KV_TT // P) + tb,
                                     dh * 512:(dh + 1) * 512], vps[:])

            # ---- Phase B: per q-tile attention + output projection ----
            with tc.tile_pool(name="pb_sb", bufs=2) as pb_sb, \
                 tc.tile_pool(name="pb_pan", bufs=2) as pb_pan, \
                 tc.tile_pool(name="mm_ps", bufs=2, space="PSUM") as mm_ps, \
                 tc.tile_pool(name="s_ps", bufs=2, space="PSUM") as s_ps, \
                 tc.tile_pool(name="y_ps", bufs=2, space="PSUM") as y_ps, \
                 tc.tile_pool(name="d_ps", bufs=1, space="PSUM") as d_ps:
                for i in range(NT):
                    nkb = i + 1  # local key blocks for this q-tile
                    q0 = i * TQ

                    # Q projection (scale folded into w_qT host-side).
                    # Per-po DMAs: a single 3-D DMA would need a 128*T*2B
                    # middle-dim stride (1 MiB at T=4096), which overflows a
                    # descriptor stride field and faults the device.
                    xq = pb_sb.tile([P, DB, TQ], bf, tag="xq")
                    for po in range(DB):
                        nc.sync.dma_start(
                            xq[:, po, :], xT_r[:, po, q0:q0 + TQ])
                    qT = pb_sb.tile([P, DB, TQ], bf, tag="qT")
                    for do in range(DB):
                        qps = mm_ps.tile([P, TQ], f32, tag="mm")
                        for di in range(DB):
                            nc.tensor.matmul(
                                qps[:],
                                wq_sb[:, di, do * P:(do + 1) * P],
                                xq[:, di, :],
                                start=(di == 0), stop=(di == DB - 1))
                        nc.vector.tensor_copy(qT[:, do, :], qps[:])

                    # S^T blocks -> exp -> (mask) -> panel; denominators
                    panel = pb_pan.tile([P, NT, TQ], bf, tag="panel")
                    dps = d_ps.tile([1, TQ], f32, tag="den")
                    for j in range(nkb):
                        sps = s_ps.tile([P, TQ], f32, tag="s")
                        for di in range(DB):
                            nc.tensor.matmul(
                                sps[:],
                                kT_sb[:, di, j * P:(j + 1) * P],
                                qT[:, di, :],
                                start=(di == 0), stop=(di == DB - 1))
                        nc.scalar.activation(
                            panel[:, j, :], sps[:],
                            mybir.ActivationFunctionType.Exp)
                        if j == nkb - 1:
                            nc.vector.tensor_mul(
                                out=panel[:, j, :], in0=panel[:, j, :],
                                in1=mask_sb[:])
                        nc.tensor.matmul(
                            dps[:], ones_sb[:], panel[:, j, :],
                            start=(j == 0), stop=(j == nkb - 1))
                    dstage = pb_sb.tile([1, TQ], f32, tag="dstage")
                    nc.vector.tensor_copy(dstage[:], dps[:])
                    nc.sync.dma_start(denom[i:i + 1, :], dstage[0:1, :])

                    # y^T[dout, q] += V[k, dout].T @ expS^T[k, q]
                    yT = pb_sb.tile([P, DB, TQ], bf, tag="yT")
                    for do in range(DB):
                        yps = y_ps.tile([P, TQ], f32, tag="y")
                        for j in range(nkb):
                            nc.tensor.matmul(
                                yps[:],
                                v_sb[:, j, do * P:(do + 1) * P],
                                panel[:, j, :],
                                start=(j == 0), stop=(j == nkb - 1))
                        nc.vector.tensor_copy(yT[:, do, :], yps[:])

                    # out^T[dout, q] += W_o^T[din, dout].T @ y^T[din, q]
                    for do in range(DB):
                        ops = mm_ps.tile([P, TQ], f32, tag="mm")
                        for di in range(DB):
                            nc.tensor.matmul(
                                ops[:],
                                wo_sb[:, di, do * P:(do + 1) * P],
                                yT[:, di, :],
                                start=(di == 0), stop=(di == DB - 1))
                        ostage = pb_sb.tile([P, TQ], f32, tag="ostage")
                        nc.vector.tensor_copy(ostage[:], ops[:])
                        nc.sync.dma_start(outT_r[:, do, q0:q0 + TQ], ostage[:])

    nc.compile()
    return nc


def _prepare_core_inputs(x, W_q, W_k, W_v, W_o, T, TQ):
    """Host-side shard prep. Returns list of 8 in_maps (bf16 ndarrays)."""
    B = x.shape[0]
    scale = 1.0 / np.sqrt(np.float32(D))

    w_qT = np.ascontiguousarray((W_q.T * scale)).astype(BF16)
    w_kT = np.ascontiguousarray(W_k.T).astype(BF16)
    w_vT = np.ascontiguousarray(W_v.T).astype(BF16)
    w_oT = np.ascontiguousarray(W_o.T).astype(BF16)

    # Diagonal masks per parity: mask[k, q] = 1 if k + 128*h <= q
    k_idx = np.arange(P)[:, None]
    q_idx = np.arange(TQ)[None, :]
    masks = [
        (k_idx + P * h <= q_idx).astype(np.float32).astype(BF16)
        for h in (0, 1)
    ]

    in_maps = []
    for b in range(B):
        xb = x[b]                                   # [T, D] fp32
        xT = np.ascontiguousarray(xb.T).astype(BF16)  # [D, T]
        # parity gather of 128-wide key blocks
        xblk = xT.reshape(D, T // (2 * P), 2, P)      # [D, n, parity, 128]
        for h in (0, 1):
            xT_kv = np.ascontiguousarray(
                xblk[:, :, h, :].reshape(D, T // 2))
            in_maps.append({
                "xT": xT, "xT_kv": xT_kv,
                "w_qT": w_qT, "w_kT": w_kT, "w_vT": w_vT, "w_oT": w_oT,
                "mask": masks[h],
            })
    return in_maps


def _merge(results, B, T):
    """Host merge: (out0+out1)/(d0+d1) per batch, back to [B, T, D] fp32."""
    out = np.empty((B, T, D), dtype=np.float32)
    for b in range(B):
        o0 = results[2 * b]["outT"]
        o1 = results[2 * b + 1]["outT"]
        d0 = results[2 * b]["denom"].reshape(T)
        d1 = results[2 * b + 1]["denom"].reshape(T)
        out[b] = ((o0 + o1) / (d0 + d1)[None, :]).T
    return out


def kernel(x, W_q, W_k, W_v, W_o):
    from concourse.bass_utils import run_bass_kernel_spmd

    x = np.asarray(x)
    B, T, d = x.shape
    assert d == D
    TQ = 256

    key = (T, TQ)
    if key not in _PROGRAM_CACHE:
        _PROGRAM_CACHE[key] = build_program(T, TQ)
    nc = _PROGRAM_CACHE[key]

    in_maps = _prepare_core_inputs(
        np.asarray(x, np.float32), np.asarray(W_q, np.float32),
        np.asarray(W_k, np.float32), np.asarray(W_v, np.float32),
        np.asarray(W_o, np.float32), T, TQ)
    res = run_bass_kernel_spmd(nc, in_maps, list(range(2 * B)))
    return _merge(res.results, B, T)
